# revision 9
# baseline (speedup 1.0000x reference)
"""Distributed Trainium2 kernel for the ABS-MAE partial-label loss.

Math: for p = softmax(outputs, axis=1) and eye the CxC identity,
    sum_k |p[n,k] - eye[j,k]| = (1 - p[n,j]) + |p[n,j] - 1| = 2 - 2*p[n,j]
so with conf = label_confidence[index] (rows of conf sum to 1),
    loss_mean = (1/N) * sum_n sum_j conf[n,j] * (2 - 2*p[n,j])
              = 2 - (2/N) * sum_n <p[n], conf[n]>.

Sharding (8 cores): label_confidence is row-sharded (6250 rows/core) and
the batch is sharded by ownership — core c handles exactly the batch items
whose index falls in its table shard (padded to K slots), so no cross-core
row movement is needed. Each core receives its owned batch rows packed with
the ownership mask ([K, C+1] input), plus local table indices. On device:
indirect-DMA gather of conf rows, e = exp(x) with row-sum accumulation
(logits are N(0,1): no max shift needed), rowdot = <e, conf>, w = mask /
sumexp, partial = matmul(rowdot, w), out_c = partial + 2/8.  Unsharding
sums the 8 partials: sum_c out_c = 2 - (2/N) * sum_n <p_n, conf_n>.
"""

import numpy as np

import concourse.bass as bass
import concourse.bacc as bacc
import concourse.mybir as mybir
import concourse.tile as tile
from concourse.bass_utils import run_bass_kernel_spmd

N = 128          # batch
C = 1000         # classes
NUM_DATA = 50000 # table rows
CORES = 8
ROWS = NUM_DATA // CORES  # 6250 per-core table shard

_nc_cache = {}
LAST_RESULTS = None  # BassKernelResults from the most recent run (for test harness)


def _build(K):
    W = C + 8  # packed input: [x | mask | pad]
    f32 = mybir.dt.float32
    i32 = mybir.dt.int32
    nc = bacc.Bacc(
        "TRN2", target_bir_lowering=False, debug=False, num_devices=CORES
    )

    xall_ext = nc.dram_tensor("xall", [K, W], f32, kind="ExternalInput")
    t_ext = nc.dram_tensor("table", [ROWS, C], f32, kind="ExternalInput")
    gidx_ext = nc.dram_tensor("gidx", [K, 1], i32, kind="ExternalInput")
    out_ext = nc.dram_tensor("out", [1, 1], f32, kind="ExternalOutput")

    with tile.TileContext(nc) as tc:
        with (
            tc.tile_pool(name="sbuf", bufs=1) as sb,
            tc.tile_pool(name="psum", bufs=1, space="PSUM") as ps,
        ):
            # scratch columns: 0=sumexp 1=rowdot 2=recip 3=w 4=warm-in 5=final 6=warm-out
            sml = sb.tile([K, 8], f32)

            # warm the ACT exp table while input DMAs are in flight
            nc.vector.memset(sml[0:1, 4:5], 0.0)
            nc.scalar.activation(
                out=sml[0:1, 6:7],
                in_=sml[0:1, 4:5],
                func=mybir.ActivationFunctionType.Exp,
            )

            # ---- loads: gather index first (it gates the indirect DMA chain) ----
            gidx = sb.tile([K, 1], i32)
            nc.sync.dma_start(out=gidx[:], in_=gidx_ext[:])
            xall = sb.tile([K, W], f32)
            nc.sync.dma_start(out=xall[:], in_=xall_ext[:])

            # ---- gather conf rows for the owned batch items ----
            conf = sb.tile([K, C], f32)
            nc.gpsimd.indirect_dma_start(
                out=conf[:],
                out_offset=None,
                in_=t_ext[:],
                in_offset=bass.IndirectOffsetOnAxis(ap=gidx[:, :1], axis=0),
            )

            # ---- e = exp(x) with per-row sums ----
            e = sb.tile([K, C], f32)
            nc.scalar.activation(
                out=e[:],
                in_=xall[:, 0:C],
                func=mybir.ActivationFunctionType.Exp,
                bias=0.0,
                scale=1.0,
                accum_out=sml[:, 0:1],
            )

            # ---- w = mask / sumexp  (mask pre-scaled by -2/N on host) ----
            nc.vector.reciprocal(out=sml[:, 2:3], in_=sml[:, 0:1])
            nc.vector.tensor_mul(sml[:, 3:4], xall[:, C : C + 1], sml[:, 2:3])

            # ---- rowdot = <e, conf> (prod computed in place over e) ----
            nc.vector.tensor_mul(e[:], e[:], conf[:])
            nc.vector.reduce_sum(
                out=sml[:, 1:2], in_=e[:], axis=mybir.AxisListType.X
            )

            # ---- partial = sum_rows rowdot*w on PE; out_c = partial + 2/8 ----
            acc = ps.tile([1, 1], f32)
            nc.tensor.matmul(
                out=acc[:], lhsT=sml[:, 1:2], rhs=sml[:, 3:4], start=True, stop=True
            )
            nc.scalar.activation(
                out=sml[0:1, 5:6],
                in_=acc[:],
                func=mybir.ActivationFunctionType.Copy,
                bias=2.0 / CORES,
                scale=1.0,
            )
            nc.sync.dma_start(out=out_ext[:], in_=sml[0:1, 5:6])

    nc.compile()
    return nc


def _get_nc(K):
    if K not in _nc_cache:
        _nc_cache[K] = _build(K)
    return _nc_cache[K]


def kernel(outputs, label_confidence, index):
    global LAST_RESULTS
    outputs = np.ascontiguousarray(np.asarray(outputs, dtype=np.float32))
    label_confidence = np.ascontiguousarray(
        np.asarray(label_confidence, dtype=np.float32)
    )
    idx = np.asarray(index).astype(np.int64).reshape(N)

    owner = idx // ROWS
    counts = np.bincount(owner, minlength=CORES)
    K = 32
    while K < int(counts.max()):
        K *= 2
    W = C + 8
    nc = _get_nc(K)

    in_maps = []
    for c in range(CORES):
        rows = np.nonzero(owner == c)[0]
        n_own = len(rows)
        rows_p = np.concatenate([rows, np.zeros(K - n_own, dtype=rows.dtype)])
        gidx = (idx[rows_p] - c * ROWS).astype(np.int32)
        gidx[n_own:] = 0
        xall = np.zeros((K, W), dtype=np.float32)
        xall[:, 0:C] = outputs[rows_p]
        xall[0:n_own, C] = -2.0 / N  # pre-scaled ownership mask
        in_maps.append(
            {
                "xall": xall,
                "table": label_confidence[c * ROWS : (c + 1) * ROWS],
                "gidx": gidx.reshape(K, 1),
            }
        )
    LAST_RESULTS = run_bass_kernel_spmd(nc, in_maps, core_ids=list(range(CORES)))
    total = np.float32(0.0)
    for c in range(CORES):
        total += np.float32(LAST_RESULTS.results[c]["out"][0, 0])
    return np.asarray(total, dtype=np.float32).reshape(())


# revision 11
# speedup vs baseline: 1.1761x; 1.1761x over previous
"""Distributed Trainium2 kernel for the ABS-MAE partial-label loss.

Math: for p = softmax(outputs, axis=1) and eye the CxC identity,
    sum_k |p[n,k] - eye[j,k]| = (1 - p[n,j]) + |p[n,j] - 1| = 2 - 2*p[n,j]
so with conf = label_confidence[index] (rows of conf sum to 1),
    loss_mean = (1/N) * sum_n sum_j conf[n,j] * (2 - 2*p[n,j])
              = 2 - (2/N) * sum_n <p[n], conf[n]>.

Sharding (8 cores): label_confidence is row-sharded (6250 rows/core) and
the batch is sharded by ownership — core c handles exactly the batch items
whose index falls in its table shard (padded to K slots), so no cross-core
row movement is needed.

Device layout: each logical row is split across Q = 128/K partitions so all
128 SBUF partitions stay busy (ACT/DVE time scales with free-dim length,
not partition count).  The conf gather is one indirect DMA over the table
viewed as [ROWS*Q, C/Q] with host-expanded per-quarter indices.  Per row:
e = exp(x) (logits are N(0,1), no max-shift needed) with per-partition sums
accumulated by the activation; a one-hot selection matmul group-sums the
partition sums back to K logical rows for the softmax normalizer; rowdot =
<e, conf> per partition, group-summed the same way; the final partial is a
[K]-dot on the TensorEngine of rowdot against mask/sumexp (mask pre-scaled
by -2/N on host; 0 on pad slots).  Each core outputs
    out_c = 2/8 - (2/N) * sum_own <p, conf>,
and unsharding sums the 8 partials:
    sum_c out_c = 2 - (2/N) * sum_n <p_n, conf_n> = loss_mean.
"""

import numpy as np

import concourse.bass as bass
import concourse.bacc as bacc
import concourse.mybir as mybir
import concourse.tile as tile
from concourse.bass_utils import run_bass_kernel_spmd

N = 128          # batch
C = 1000         # classes
NUM_DATA = 50000 # table rows
CORES = 8
ROWS = NUM_DATA // CORES  # 6250 per-core table shard
P = 128          # SBUF partitions

_nc_cache = {}
LAST_RESULTS = None  # BassKernelResults from the most recent run (for test harness)


def _build(K):
    Q = P // K        # partitions per logical row
    CQ = C // Q       # columns per partition
    W = CQ + 1 + K    # packed input: [x_q | mask | sel]
    f32 = mybir.dt.float32
    i32 = mybir.dt.int32
    EXP = mybir.ActivationFunctionType.Exp
    CPY = mybir.ActivationFunctionType.Copy
    nc = bacc.Bacc(
        "TRN2", target_bir_lowering=False, debug=False, num_devices=CORES
    )

    xall_ext = nc.dram_tensor("xall", [P, W], f32, kind="ExternalInput")
    t_ext = nc.dram_tensor("table", [ROWS * Q, CQ], f32, kind="ExternalInput")
    gidx_ext = nc.dram_tensor("gidx", [P, 1], i32, kind="ExternalInput")
    out_ext = nc.dram_tensor("out", [1, 1], f32, kind="ExternalOutput")

    with tile.TileContext(nc) as tc:
        with (
            tc.tile_pool(name="sbuf", bufs=1) as sb,
            tc.tile_pool(name="psum", bufs=1, space="PSUM") as ps,
        ):
            # scratch: col0 = per-partition sumexp, col1 = per-partition rowdot,
            # col2 = final out, col3 = warm-exp sink
            sml = sb.tile([P, 4], f32)

            # dummy activation: pull the ACT exp table in while DMAs fly
            nc.vector.memset(sml[0:1, 2:3], 0.0)
            nc.scalar.activation(out=sml[0:1, 3:4], in_=sml[0:1, 2:3], func=EXP)

            # ---- loads (gather index first: it gates the indirect DMA) ----
            gidx = sb.tile([P, 1], i32)
            nc.sync.dma_start(out=gidx[:], in_=gidx_ext[:])
            xall = sb.tile([P, W], f32)
            nc.sync.dma_start(out=xall[:], in_=xall_ext[:])

            # ---- gather conf quarter-rows for the owned batch items ----
            conf = sb.tile([P, CQ], f32)
            nc.gpsimd.indirect_dma_start(
                out=conf[:],
                out_offset=None,
                in_=t_ext[:],
                in_offset=bass.IndirectOffsetOnAxis(ap=gidx[:, :1], axis=0),
            )

            # ---- e = exp(x) with per-partition sums ----
            e = sb.tile([P, CQ], f32)
            nc.scalar.activation(
                out=e[:],
                in_=xall[:, 0:CQ],
                func=EXP,
                bias=0.0,
                scale=1.0,
                accum_out=sml[:, 0:1],
            )

            # ---- sumexp per logical row via selection matmul; w = mask/sumexp ----
            sel = xall[:, CQ + 1 : CQ + 1 + K]
            s32p = ps.tile([K, 1], f32)
            nc.tensor.matmul(
                out=s32p[:], lhsT=sel, rhs=sml[:, 0:1], start=True, stop=True
            )
            rw = sb.tile([K, 2], f32)  # col0 = 1/sumexp, col1 = w
            nc.vector.reciprocal(out=rw[:, 0:1], in_=s32p[:])
            nc.vector.tensor_mul(rw[:, 1:2], xall[0:K, CQ : CQ + 1], rw[:, 0:1])

            # ---- rowdot per partition (prod in place), group-sum, total ----
            nc.vector.tensor_mul(e[:], e[:], conf[:])
            nc.vector.reduce_sum(
                out=sml[:, 1:2], in_=e[:], axis=mybir.AxisListType.X
            )
            r32p = ps.tile([K, 1], f32)
            nc.tensor.matmul(
                out=r32p[:], lhsT=sel, rhs=sml[:, 1:2], start=True, stop=True
            )
            r32 = sb.tile([K, 1], f32)
            nc.vector.tensor_copy(out=r32[:], in_=r32p[:])
            acc = ps.tile([1, 1], f32)
            nc.tensor.matmul(
                out=acc[:], lhsT=r32[:], rhs=rw[:, 1:2], start=True, stop=True
            )

            # ---- out_c = partial + 2/CORES ----
            nc.scalar.activation(
                out=sml[0:1, 2:3], in_=acc[:], func=CPY, bias=2.0 / CORES, scale=1.0
            )
            nc.sync.dma_start(out=out_ext[:], in_=sml[0:1, 2:3])

    nc.compile()
    return nc


def _get_nc(K):
    if K not in _nc_cache:
        _nc_cache[K] = _build(K)
    return _nc_cache[K]


def kernel(outputs, label_confidence, index):
    global LAST_RESULTS
    outputs = np.ascontiguousarray(np.asarray(outputs, dtype=np.float32))
    label_confidence = np.ascontiguousarray(
        np.asarray(label_confidence, dtype=np.float32)
    )
    idx = np.asarray(index).astype(np.int64).reshape(N)

    owner = idx // ROWS
    counts = np.bincount(owner, minlength=CORES)
    K = 32
    while K < int(counts.max()):
        K *= 2
    Q = P // K
    CQ = C // Q
    W = CQ + 1 + K
    nc = _get_nc(K)

    sel = np.zeros((P, K), dtype=np.float32)
    sel[np.arange(P), np.arange(P) // Q] = 1.0

    in_maps = []
    for c in range(CORES):
        rows = np.nonzero(owner == c)[0]
        n_own = len(rows)
        rows_p = np.concatenate([rows, np.zeros(K - n_own, dtype=rows.dtype)])
        # per-quarter gather indices into the [ROWS*Q, CQ] table view
        g = (idx[rows_p] - c * ROWS).astype(np.int64)
        g[n_own:] = 0
        gidx = (g[:, None] * Q + np.arange(Q)[None, :]).astype(np.int32).reshape(P, 1)
        mask = np.full(K, -2.0 / N, dtype=np.float32)
        mask[n_own:] = 0.0

        xall = np.zeros((P, W), dtype=np.float32)
        xall[:, 0:CQ] = outputs[rows_p].reshape(P, CQ)
        xall[0:K, CQ] = mask
        xall[:, CQ + 1 :] = sel
        in_maps.append(
            {
                "xall": xall,
                "table": label_confidence[c * ROWS : (c + 1) * ROWS].reshape(
                    ROWS * Q, CQ
                ),
                "gidx": gidx,
            }
        )
    LAST_RESULTS = run_bass_kernel_spmd(nc, in_maps, core_ids=list(range(CORES)))
    total = np.float32(0.0)
    for c in range(CORES):
        total += np.float32(LAST_RESULTS.results[c]["out"][0, 0])
    return np.asarray(total, dtype=np.float32).reshape(())
